# revision 16
# baseline (speedup 1.0000x reference)
"""Banded (lookahead) cross-attention on 8 Trainium2 NeuronCores.

Reference computation (B=4, T=2048, D=1024, H=16, hd=64):
    Q = query @ Wq.T + bq ; K = key_value @ Wk.T + bk ; V = key_value @ Wv.T + bv
    scores = Q K^T / sqrt(hd), masked to j <= i + lookahead
    out = softmax(scores) V, concat heads, @ Wo.T + bo

Sharding: 8 cores = (batch b = c//2) x (head-half = c%2, 8 heads each).
Each core computes a full [T, D] partial of the output projection for its
8 heads; host sums the two partials per batch and adds bo.

v4: one flat pool scope (PSUM = pp(2) + st(4) + ot(2) banks) and a
need-driven scheduler that interleaves projection/output-projection
matmul groups into the attention stream so the PE never drains:
  A/B: Q^T,K^T bf16; bias-add fused into the psum drain (ACT in the
     prologue while Scalar is idle, DVE thereafter) -> qT/kT bf16
  C: V + bv via DVE tensor_tensor add with a broadcast-bias tile,
     strided into v_sb bf16 [128, 8*65]; ones columns via memset
  D: per (i-chunk, head) banded attention, S^T layout [j, i].  Pairs =
     (max-delta block, zero-delta block) so each exp ACT covers exactly
     the valid columns.  exp -> pt bf16; wedge masks = DVE bf16 multiplies;
     denominator row -> DVE copy to SBUF -> reciprocal_approx_fast ->
     gpsimd partition broadcast -> DVE multiply -> aT bf16.
  E: outT = Wo_s^T.T @ A^T, per i-chunk, interleaved one i-chunk behind.
Host: out[b] = (outT[2b] + outT[2b+1]).T + bo
"""

import sys

for _p in ("/opt/trn_rl_repo", "/opt/pypackages"):
    if _p not in sys.path:
        sys.path.append(_p)

import numpy as np
import ml_dtypes

import concourse.bass as bass
import concourse.tile as tile
from concourse import bacc, mybir
from concourse.bass_utils import run_bass_kernel_spmd

F32 = mybir.dt.float32
BF16 = mybir.dt.bfloat16
AF = mybir.ActivationFunctionType
MUL = mybir.AluOpType.mult
ADD = mybir.AluOpType.add

B, T, D = 4, 2048, 1024
H, HD = 16, 64
H_LOC = 8                    # heads per core
E_LOC = H_LOC * HD           # 512 projected dims per core
NJB = T // 128               # 16 j-blocks
NIC = T // 512               # 4 i-chunks
NDT = D // 128               # 8 contraction tiles
NET = E_LOC // 128           # 4 e-tiles
SCALE = HD ** -0.5
VW = H_LOC * (HD + 1)        # 520 v_sb layout width
VH = HD + 1                  # 65

_CACHE = {}


def _groups(L):
    """Per i-chunk: list of (jb, delta, masked); delta = first valid column
    offset inside the 512-wide chunk (0 for dense)."""
    out = []
    deltas = set()
    for ic in range(NIC):
        i0 = 512 * ic
        lst = []
        for jb in range(NJB):
            j0 = 128 * jb
            if i0 + 511 + L < j0:
                break                          # fully masked from here on
            if j0 + 127 <= i0 + L:
                lst.append((jb, 0, False))     # dense
            else:
                d = j0 - L - i0
                lst.append((jb, max(d, 0), True))
                deltas.add(d)
        out.append(lst)
    return out, sorted(deltas)


def _pairs(lst):
    """Pair the largest-delta block with a zero-delta block so the exp ACT
    range [pair0.delta, 1024) has no unwritten-psum gap.  Order pairs by
    their max jb so early items only need early K/V tiles."""
    srt = sorted(lst, key=lambda b: -b[1])
    n = len(srt)
    prs = [(srt[i], srt[n - 1 - i]) for i in range(n // 2)]
    if n % 2:
        prs.append((srt[n // 2],))
    prs.sort(key=lambda pr: max(b[0] for b in pr))
    return prs


def _build(L):
    groups, deltas = _groups(L)
    dpos = {d: k for k, d in enumerate(deltas)}
    nmask = max(1, len(deltas))
    pairs_by_ic = [_pairs(groups[ic]) for ic in range(NIC)]
    nblocks = [len(groups[ic]) for ic in range(NIC)]

    nc = bacc.Bacc("TRN2", target_bir_lowering=False, debug=False)
    xqT = nc.dram_tensor("xqT", [D, T], BF16, kind="ExternalInput").ap()
    xkvT = nc.dram_tensor("xkvT", [D, T], BF16, kind="ExternalInput").ap()
    wqT = nc.dram_tensor("wqT", [D, E_LOC], BF16, kind="ExternalInput").ap()
    wkT = nc.dram_tensor("wkT", [D, E_LOC], BF16, kind="ExternalInput").ap()
    wvT = nc.dram_tensor("wvT", [D, E_LOC], BF16, kind="ExternalInput").ap()
    woT = nc.dram_tensor("woT", [E_LOC, D], BF16, kind="ExternalInput").ap()
    bq4 = nc.dram_tensor("bq4", [128, NET], F32, kind="ExternalInput").ap()
    bk4 = nc.dram_tensor("bk4", [128, NET], F32, kind="ExternalInput").ap()
    bvb = nc.dram_tensor("bvb", [128, E_LOC], BF16, kind="ExternalInput").ap()
    masks = nc.dram_tensor("masks", [128, nmask * 512], BF16,
                           kind="ExternalInput").ap()
    outT = nc.dram_tensor("outT", [D, T], BF16, kind="ExternalOutput").ap()

    with tile.TileContext(nc) as tc:
        with tc.tile_pool(name="small", bufs=1) as small, \
             tc.tile_pool(name="persist", bufs=1) as persist, \
             tc.tile_pool(name="slabs", bufs=1) as slabs, \
             tc.tile_pool(name="ptp", bufs=10) as pt_pool, \
             tc.tile_pool(name="dv", bufs=2) as dv_pool, \
             tc.tile_pool(name="stg", bufs=2) as stg_pool, \
             tc.tile_pool(name="pp", bufs=2, space="PSUM") as pp, \
             tc.tile_pool(name="sps", bufs=2, space="PSUM") as sps, \
             tc.tile_pool(name="ops", bufs=2, space="PSUM") as ops:

            # ---- SBUF tiles ----
            # per-(d, et) weight tiles: the prologue's first group only waits
            # on the et=0 slices
            wq_sb = {(d, e): slabs.tile([128, 128], BF16, tag=f"wq{d}_{e}",
                                        name=f"wq{d}_{e}")
                     for d in range(NDT) for e in range(NET)}
            wk_sb = {(d, e): slabs.tile([128, 128], BF16, tag=f"wk{d}_{e}",
                                        name=f"wk{d}_{e}")
                     for d in range(NDT) for e in range(NET)}
            wv_sb = [slabs.tile([128, E_LOC], BF16, tag=f"wv{d}", name=f"wv{d}")
                     for d in range(NDT)]
            wo_sb = [slabs.tile([128, D], BF16, tag=f"wo{e}", name=f"wo{e}")
                     for e in range(NET)]
            xq_sb = {}
            xkv_sb = {}
            for t in range(NIC):
                for d in range(NDT):
                    xq_sb[(d, t)] = slabs.tile(
                        [128, 512], BF16, tag=f"xq{d}", bufs=2,
                        name=f"xq{d}_{t}")
                    xkv_sb[(d, t)] = slabs.tile(
                        [128, 512], BF16, tag=f"xkv{d}_{t}",
                        name=f"xkv{d}_{t}")
            bq_sb = small.tile([128, NET], F32, tag="bq")
            bk_sb = small.tile([128, NET], F32, tag="bk")
            bv_sb = small.tile([128, E_LOC], BF16, tag="bvb")
            mk_sb = persist.tile([128, nmask * 512], BF16, tag="mk")

            qT = [persist.tile([128, T], BF16, tag=f"qt{i}", name=f"qt{i}")
                  for i in range(NET)]
            kT = [persist.tile([128, T], BF16, tag=f"kt{i}", name=f"kt{i}")
                  for i in range(NET)]
            v_sb = [persist.tile([128, VW], BF16, tag=f"v{i}", name=f"v{i}")
                    for i in range(NJB)]
            aT = [persist.tile([128, T], BF16, tag=f"at{i}", name=f"at{i}")
                  for i in range(NET)]

            # ---- build work list + first-need schedule ----
            items = []   # (ic, h, pair)
            for ic in range(NIC):
                for h in range(H_LOC):
                    for pr in pairs_by_ic[ic]:
                        items.append((ic, h, pr))

            def need_keys(ic, h, pair):
                et = h // 2
                ks = [("A", ic, et)]
                for jb, _, _ in pair:
                    ks.append(("B", et, (128 * jb) // 512))
                    ks.append(("C", jb))
                return ks

            proj_order = []       # keys in first-need order
            first_need = {}
            seen = set()
            for n, (ic, h, pair) in enumerate(items):
                for k in need_keys(ic, h, pair):
                    if k not in seen:
                        seen.add(k)
                        proj_order.append(k)
                        first_need[k] = n

            # ---- DMA issue order == first-need order ----
            dma_done = set()

            def dma_for(key):
                kind = key[0]
                if kind == "A":
                    t, et = key[1], key[2]
                    if f"wq_{et}" not in dma_done:
                        dma_done.add(f"wq_{et}")
                        for d in range(NDT):
                            nc.sync.dma_start(
                                wq_sb[(d, et)][:],
                                wqT[128 * d:128 * (d + 1),
                                    128 * et:128 * (et + 1)])
                    if f"xq_{t}" not in dma_done:
                        dma_done.add(f"xq_{t}")
                        for d in range(NDT):
                            nc.sync.dma_start(
                                xq_sb[(d, t)][:],
                                xqT[128 * d:128 * (d + 1),
                                    512 * t:512 * (t + 1)])
                elif kind == "B":
                    et, t = key[1], key[2]
                    if f"wk_{et}" not in dma_done:
                        dma_done.add(f"wk_{et}")
                        for d in range(NDT):
                            nc.sync.dma_start(
                                wk_sb[(d, et)][:],
                                wkT[128 * d:128 * (d + 1),
                                    128 * et:128 * (et + 1)])
                    if f"xkv_{t}" not in dma_done:
                        dma_done.add(f"xkv_{t}")
                        for d in range(NDT):
                            nc.sync.dma_start(
                                xkv_sb[(d, t)][:],
                                xkvT[128 * d:128 * (d + 1),
                                     512 * t:512 * (t + 1)])
                elif kind == "C":
                    tq = key[1] // 4
                    for dk in ("wv", f"xkv_{tq}"):
                        if dk not in dma_done:
                            dma_done.add(dk)
                            if dk == "wv":
                                for d in range(NDT):
                                    nc.sync.dma_start(
                                        wv_sb[d][:],
                                        wvT[128 * d:128 * (d + 1), :])
                            else:
                                for d in range(NDT):
                                    nc.sync.dma_start(
                                        xkv_sb[(d, tq)][:],
                                        xkvT[128 * d:128 * (d + 1),
                                             512 * tq:512 * (tq + 1)])

            nc.sync.dma_start(bq_sb[:], bq4[:])
            nc.sync.dma_start(bk_sb[:], bk4[:])
            nc.sync.dma_start(bv_sb[:], bvb[:])
            nc.sync.dma_start(mk_sb[:], masks[:])
            AHEAD = 5
            for k in proj_order:
                if first_need[k] <= AHEAD:
                    dma_for(k)
            wo_dma = [False]

            def dma_wo():
                if not wo_dma[0]:
                    wo_dma[0] = True
                    for e in range(NET):
                        nc.sync.dma_start(wo_sb[e][:],
                                          woT[128 * e:128 * (e + 1), :])
            for k in proj_order:
                dma_for(k)
                if first_need[k] > len(items) // 4:
                    dma_wo()
            dma_wo()

            # softmax-denominator ones columns (static)
            for tt in range(NJB):
                vv = v_sb[tt][:].rearrange("p (h w) -> p h w", w=VH)
                nc.vector.memset(vv[:, :, HD:VH], 1.0)

            # ---- projection-group emitters ----
            def emit_proj(key, prologue):
                kind = key[0]
                if kind == "A" or kind == "B":
                    t, et = (key[1], key[2]) if kind == "A" else (key[2], key[1])
                    wsb, xsb = (wq_sb, xq_sb) if kind == "A" else (wk_sb, xkv_sb)
                    dst = qT if kind == "A" else kT
                    bias = bq_sb if kind == "A" else bk_sb
                    ps = pp.tile([128, 512], F32, tag="pp")
                    for d in range(NDT):
                        nc.tensor.matmul(
                            ps[:], wsb[(d, et)][:], xsb[(d, t)][:],
                            start=(d == 0), stop=(d == NDT - 1))
                    out = dst[et][:, 512 * t:512 * (t + 1)]
                    if prologue:
                        nc.scalar.activation(out, ps[:], AF.Identity,
                                             bias=bias[:, et:et + 1])
                    else:
                        nc.vector.tensor_scalar_add(out, ps[:],
                                                    bias[:, et:et + 1])
                else:
                    tt = key[1]
                    tq, tc_ = tt // 4, tt % 4
                    ps = pp.tile([128, 512], F32, tag="pp")
                    for d in range(NDT):
                        nc.tensor.matmul(
                            ps[:],
                            xkv_sb[(d, tq)][:, 128 * tc_:128 * (tc_ + 1)],
                            wv_sb[d][:], start=(d == 0), stop=(d == NDT - 1))
                    vv = v_sb[tt][:].rearrange("p (h w) -> p h w", w=VH)
                    nc.vector.tensor_tensor(
                        vv[:, :, 0:HD],
                        ps[:].rearrange("p (h w) -> p h w", w=HD),
                        bv_sb[:].rearrange("p (h w) -> p h w", w=HD), ADD)

            # ---- phase D + interleaved fillers ----
            DEPTH = 5
            ot = {}
            issued = {}
            pending = {}
            e_groups = []    # (push_item, ic, do)
            cur_item = [0]
            pq = list(proj_order)

            def emit_e():
                _, ic, do = e_groups.pop(0)
                ps = pp.tile([128, 512], F32, tag="pp")
                for e in range(NET):
                    nc.tensor.matmul(
                        ps[:], wo_sb[e][:, 128 * do:128 * (do + 1)],
                        aT[e][:, 512 * ic:512 * (ic + 1)],
                        start=(e == 0), stop=(e == NET - 1))
                o = stg_pool.tile([128, 512], BF16, tag="stg")
                nc.vector.tensor_scalar_add(o[:], ps[:], 0.0)
                nc.sync.dma_start(
                    outT[128 * do:128 * (do + 1),
                         512 * ic:512 * (ic + 1)], o[:])

            def finish_group(ic, h):
                o = ot.pop((ic, h))
                et, r0 = h // 2, 64 * (h % 2)
                # reciprocal_approx_fast's bitwise seed misreads PSUM inputs;
                # stage the denominator row through SBUF first
                dcp = dv_pool.tile([1, 512], F32, tag="dc")
                nc.vector.tensor_scalar_add(dcp[:], o[64:65, :], 0.0)
                r = dv_pool.tile([1, 512], F32, tag="r")
                nc.vector.reciprocal_approx_fast(r[:], dcp[:])
                rb = dv_pool.tile([64, 512], F32, tag="rb")
                nc.gpsimd.partition_broadcast(rb[:], r[:])
                nc.vector.tensor_tensor(
                    aT[et][r0:r0 + 64, 512 * ic:512 * (ic + 1)],
                    o[0:64, :], rb[:], MUL)
                if h == H_LOC - 1:
                    for do in range(NDT):
                        e_groups.append((cur_item[0], ic, do))

            def emit_pv(n):
                ic, h, pair = items[n]
                pt = pending.pop(n)
                # ascending delta within the pair: group's first-issued PV is
                # always a delta-0 block, so start=True covers [0:512)
                for s in sorted(range(len(pair)), key=lambda q: pair[q][1]):
                    jb, dlt, msk = pair[s]
                    k = issued[(ic, h)] = issued.get((ic, h), 0) + 1
                    nc.tensor.matmul(
                        ot[(ic, h)][:, dlt:512],
                        v_sb[jb][:, VH * h:VH * h + VH],
                        pt[:, 512 * s + dlt:512 * (s + 1)],
                        start=(k == 1), stop=(k == nblocks[ic]),
                        skip_group_check=True)
                if issued[(ic, h)] == nblocks[ic]:
                    finish_group(ic, h)

            # prologue: groups needed by the first AHEAD items
            for k in list(pq):
                if first_need[k] <= AHEAD:
                    emit_proj(k, prologue=True)
                    pq.remove(k)

            for n, (ic, h, pair) in enumerate(items):
                cur_item[0] = n
                # dependency-driven projection groups (with lookahead)
                while pq and first_need[pq[0]] <= n + AHEAD:
                    emit_proj(pq.pop(0), prologue=False)
                if e_groups and n - e_groups[0][0] >= 2:
                    emit_e()
                if (ic, h) not in ot:
                    ot[(ic, h)] = ops.tile([65, 512], F32, tag="ot",
                                           name=f"ot{ic}_{h}")
                et, r0 = h // 2, 64 * (h % 2)
                st = sps.tile([128, 1024], F32, tag="st")
                for s, (jb, dlt, msk) in enumerate(pair):
                    nc.tensor.matmul(
                        st[:, 512 * s + dlt:512 * (s + 1)],
                        kT[et][r0:r0 + 64, 128 * jb:128 * (jb + 1)],
                        qT[et][r0:r0 + 64, 512 * ic + dlt:512 * (ic + 1)],
                        start=True, stop=True)
                pt = pt_pool.tile([128, 1024], BF16, tag="pt", bufs=10)
                lo = pair[0][1]
                hi = 512 * (len(pair) - 1) + 512
                nc.scalar.activation(pt[:, lo:hi], st[:, lo:hi],
                                     AF.Exp, scale=SCALE)
                for s, (jb, dlt, msk) in enumerate(pair):
                    if msk:
                        k = dpos[128 * jb - L - 512 * ic]
                        nc.vector.tensor_tensor(
                            pt[:, 512 * s + dlt:512 * s + dlt + 128],
                            pt[:, 512 * s + dlt:512 * s + dlt + 128],
                            mk_sb[:, 512 * k + dlt:512 * k + dlt + 128], MUL)
                pending[n] = pt
                if n >= DEPTH:
                    emit_pv(n - DEPTH)
            while pq:
                emit_proj(pq.pop(0), prologue=False)
            for n in range(max(0, len(items) - DEPTH), len(items)):
                emit_pv(n)
            while e_groups:
                emit_e()

    nc.compile()
    return nc, deltas


def _prep_core(query, key_value, Wq, bq, Wk, bk, Wv, bv, Wo, c, deltas, L):
    b, half = c // 2, c % 2
    hs = E_LOC * half
    f32, bf16 = np.float32, ml_dtypes.bfloat16
    xqT = np.ascontiguousarray(query[b].T).astype(bf16)
    xkvT = np.ascontiguousarray(key_value[b].T).astype(bf16)
    wqT = np.ascontiguousarray(Wq[hs:hs + E_LOC].T).astype(bf16)
    wkT = np.ascontiguousarray(Wk[hs:hs + E_LOC].T).astype(bf16)
    wvT = np.ascontiguousarray(Wv[hs:hs + E_LOC].T).astype(bf16)
    woT = np.ascontiguousarray(Wo[:, hs:hs + E_LOC].T).astype(bf16)
    bq4 = np.ascontiguousarray(bq[hs:hs + E_LOC].reshape(NET, 128).T, dtype=f32)
    bk4 = np.ascontiguousarray(bk[hs:hs + E_LOC].reshape(NET, 128).T, dtype=f32)
    bvb = np.broadcast_to(bv[hs:hs + E_LOC].reshape(1, E_LOC),
                          (128, E_LOC)).astype(bf16)
    nmask = max(1, len(deltas))
    masks = np.zeros((128, nmask * 512), dtype=np.float32)
    jr = np.arange(128)[:, None]
    ir = np.arange(512)[None, :]
    for k, d in enumerate(deltas):
        masks[:, 512 * k:512 * (k + 1)] = (jr <= ir - d).astype(f32)
    masks = masks.astype(bf16)
    return {"xqT": xqT, "xkvT": xkvT, "wqT": wqT, "wkT": wkT, "wvT": wvT,
            "woT": woT, "bq4": bq4, "bk4": bk4, "bvb": bvb, "masks": masks}


def kernel(query, key_value, Wq, bq, Wk, bk, Wv, bv, Wo, bo, lookahead,
           _trace=False):
    L = int(lookahead)
    if L not in _CACHE:
        _CACHE[L] = _build(L)
    nc, deltas = _CACHE[L]

    args = [np.asarray(a, dtype=np.float32) for a in
            (query, key_value, Wq, bq, Wk, bk, Wv, bv, Wo)]
    in_maps = [_prep_core(*args, c, deltas, L) for c in range(8)]
    res = run_bass_kernel_spmd(nc, in_maps, core_ids=list(range(8)),
                               trace=_trace)
    bo = np.asarray(bo, dtype=np.float32)
    out = np.empty((B, T, D), dtype=np.float32)
    for b in range(B):
        pT = (res.results[2 * b]["outT"].astype(np.float32)
              + res.results[2 * b + 1]["outT"].astype(np.float32))
        out[b] = pT.T + bo[None, :]
    if _trace:
        kernel.last_exec_time_ns = res.exec_time_ns
    return out


# revision 19
# speedup vs baseline: 1.0316x; 1.0316x over previous
"""Banded (lookahead) cross-attention on 8 Trainium2 NeuronCores.

Reference computation (B=4, T=2048, D=1024, H=16, hd=64):
    Q = query @ Wq.T + bq ; K = key_value @ Wk.T + bk ; V = key_value @ Wv.T + bv
    scores = Q K^T / sqrt(hd), masked to j <= i + lookahead
    out = softmax(scores) V, concat heads, @ Wo.T + bo

Sharding: 8 cores = (batch b = c//2) x (head-half = c%2, 8 heads each).
Each core computes a full [T, D] partial of the output projection for its
8 heads; host sums the two partials per batch and adds bo.

v4: one flat pool scope (PSUM = pp(2) + st(4) + ot(2) banks) and a
need-driven scheduler that interleaves projection/output-projection
matmul groups into the attention stream so the PE never drains:
  A/B: Q^T,K^T bf16; bias-add fused into the psum drain (ACT in the
     prologue while Scalar is idle, DVE thereafter) -> qT/kT bf16
  C: V + bv via DVE tensor_tensor add with a broadcast-bias tile,
     strided into v_sb bf16 [128, 8*65]; ones columns via memset
  D: per (i-chunk, head) banded attention, S^T layout [j, i].  Pairs =
     (max-delta block, zero-delta block) so each exp ACT covers exactly
     the valid columns.  exp -> pt bf16; wedge masks = DVE bf16 multiplies;
     denominator row -> DVE copy to SBUF -> reciprocal_approx_fast ->
     gpsimd partition broadcast -> DVE multiply -> aT bf16.
  E: outT = Wo_s^T.T @ A^T, per i-chunk, interleaved one i-chunk behind.
Host: out[b] = (outT[2b] + outT[2b+1]).T + bo
"""

import sys

for _p in ("/opt/trn_rl_repo", "/opt/pypackages"):
    if _p not in sys.path:
        sys.path.append(_p)

import numpy as np
import ml_dtypes

import concourse.bass as bass
import concourse.tile as tile
from concourse import bacc, mybir
from concourse.bass_utils import run_bass_kernel_spmd

F32 = mybir.dt.float32
BF16 = mybir.dt.bfloat16
AF = mybir.ActivationFunctionType
MUL = mybir.AluOpType.mult
ADD = mybir.AluOpType.add

B, T, D = 4, 2048, 1024
H, HD = 16, 64
H_LOC = 8                    # heads per core
E_LOC = H_LOC * HD           # 512 projected dims per core
NJB = T // 128               # 16 j-blocks
NIC = T // 512               # 4 i-chunks
NDT = D // 128               # 8 contraction tiles
NET = E_LOC // 128           # 4 e-tiles
SCALE = HD ** -0.5
VW = H_LOC * (HD + 1)        # 520 v_sb layout width
VH = HD + 1                  # 65

_CACHE = {}


def _groups(L):
    """Per i-chunk: list of (jb, delta, masked); delta = first valid column
    offset inside the 512-wide chunk (0 for dense)."""
    out = []
    deltas = set()
    for ic in range(NIC):
        i0 = 512 * ic
        lst = []
        for jb in range(NJB):
            j0 = 128 * jb
            if i0 + 511 + L < j0:
                break                          # fully masked from here on
            if j0 + 127 <= i0 + L:
                lst.append((jb, 0, False))     # dense
            else:
                d = j0 - L - i0
                lst.append((jb, max(d, 0), True))
                deltas.add(d)
        out.append(lst)
    return out, sorted(deltas)


def _pairs(lst):
    """Pair the largest-delta block with a zero-delta block so the exp ACT
    range [pair0.delta, 1024) has no unwritten-psum gap.  Order pairs by
    their max jb so early items only need early K/V tiles."""
    srt = sorted(lst, key=lambda b: -b[1])
    n = len(srt)
    prs = [(srt[i], srt[n - 1 - i]) for i in range(n // 2)]
    if n % 2:
        prs.append((srt[n // 2],))
    prs.sort(key=lambda pr: max(b[0] for b in pr))
    return prs


def _build(L):
    groups, deltas = _groups(L)
    dpos = {d: k for k, d in enumerate(deltas)}
    nmask = max(1, len(deltas))
    pairs_by_ic = [_pairs(groups[ic]) for ic in range(NIC)]
    nblocks = [len(groups[ic]) for ic in range(NIC)]

    nc = bacc.Bacc("TRN2", target_bir_lowering=False, debug=False)
    xqT = nc.dram_tensor("xqT", [D, T], BF16, kind="ExternalInput").ap()
    xkvT = nc.dram_tensor("xkvT", [D, T], BF16, kind="ExternalInput").ap()
    wqT = nc.dram_tensor("wqT", [D, E_LOC], BF16, kind="ExternalInput").ap()
    wkT = nc.dram_tensor("wkT", [D, E_LOC], BF16, kind="ExternalInput").ap()
    wvT = nc.dram_tensor("wvT", [D, E_LOC], BF16, kind="ExternalInput").ap()
    woT = nc.dram_tensor("woT", [E_LOC, D], BF16, kind="ExternalInput").ap()
    bq4 = nc.dram_tensor("bq4", [128, NET], F32, kind="ExternalInput").ap()
    bk4 = nc.dram_tensor("bk4", [128, NET], F32, kind="ExternalInput").ap()
    bvb = nc.dram_tensor("bvb", [128, E_LOC], BF16, kind="ExternalInput").ap()
    masks = nc.dram_tensor("masks", [128, nmask * 512], BF16,
                           kind="ExternalInput").ap()
    outT = nc.dram_tensor("outT", [D, T], BF16, kind="ExternalOutput").ap()

    with tile.TileContext(nc) as tc:
        with tc.tile_pool(name="small", bufs=1) as small, \
             tc.tile_pool(name="persist", bufs=1) as persist, \
             tc.tile_pool(name="slabs", bufs=1) as slabs, \
             tc.tile_pool(name="ptp", bufs=10) as pt_pool, \
             tc.tile_pool(name="dv", bufs=2) as dv_pool, \
             tc.tile_pool(name="stg", bufs=2) as stg_pool, \
             tc.tile_pool(name="pp", bufs=2, space="PSUM") as pp, \
             tc.tile_pool(name="sps", bufs=2, space="PSUM") as sps, \
             tc.tile_pool(name="ops", bufs=2, space="PSUM") as ops:

            # ---- SBUF tiles ----
            wq_sb = [slabs.tile([128, E_LOC], BF16, tag=f"wq{d}", name=f"wq{d}")
                     for d in range(NDT)]
            wk_sb = [slabs.tile([128, E_LOC], BF16, tag=f"wk{d}", name=f"wk{d}")
                     for d in range(NDT)]
            wv_sb = [slabs.tile([128, E_LOC], BF16, tag=f"wv{d}", name=f"wv{d}")
                     for d in range(NDT)]
            wo_sb = [slabs.tile([128, D], BF16, tag=f"wo{e}", name=f"wo{e}")
                     for e in range(NET)]
            xq_sb = {}
            xkv_sb = {}
            for t in range(NIC):
                for d in range(NDT):
                    xq_sb[(d, t)] = slabs.tile(
                        [128, 512], BF16, tag=f"xq{d}", bufs=2,
                        name=f"xq{d}_{t}")
                    xkv_sb[(d, t)] = slabs.tile(
                        [128, 512], BF16, tag=f"xkv{d}_{t}",
                        name=f"xkv{d}_{t}")
            bq_sb = small.tile([128, NET], F32, tag="bq")
            bk_sb = small.tile([128, NET], F32, tag="bk")
            bv_sb = small.tile([128, E_LOC], BF16, tag="bvb")
            mk_sb = persist.tile([128, nmask * 512], BF16, tag="mk")

            qT = [persist.tile([128, T], BF16, tag=f"qt{i}", name=f"qt{i}")
                  for i in range(NET)]
            kT = [persist.tile([128, T], BF16, tag=f"kt{i}", name=f"kt{i}")
                  for i in range(NET)]
            v_sb = [persist.tile([128, VW], BF16, tag=f"v{i}", name=f"v{i}")
                    for i in range(NJB)]
            aT = [persist.tile([128, T], BF16, tag=f"at{i}", name=f"at{i}")
                  for i in range(NET)]

            # ---- build work list + first-need schedule ----
            items = []   # (ic, h, pair)
            for ic in range(NIC):
                for h in range(H_LOC):
                    for pr in pairs_by_ic[ic]:
                        items.append((ic, h, pr))

            def need_keys(ic, h, pair):
                et = h // 2
                ks = [("A", ic, et)]
                for jb, _, _ in pair:
                    ks.append(("B", et, (128 * jb) // 512))
                    ks.append(("C", jb))
                return ks

            proj_order = []       # keys in first-need order
            first_need = {}
            seen = set()
            for n, (ic, h, pair) in enumerate(items):
                for k in need_keys(ic, h, pair):
                    if k not in seen:
                        seen.add(k)
                        proj_order.append(k)
                        first_need[k] = n

            # ---- DMA issue order == first-need order ----
            # descriptor generation is ~0.5us per dma_start and serializes on
            # the issuing engine; round-robin the startup-critical DMAs over
            # SP / Scalar / GpSimd so the prologue's inputs land fast
            dma_done = set()
            rr = [0]
            dma_engines = [nc.sync, nc.scalar, nc.gpsimd]

            def dma(dst, src):
                eng = dma_engines[rr[0] % len(dma_engines)]
                rr[0] += 1
                eng.dma_start(dst, src)

            def dma_for(key):
                kind = key[0]
                if kind == "A":
                    t = key[1]
                    if "wq" not in dma_done:
                        dma_done.add("wq")
                        for d in range(NDT):
                            dma(wq_sb[d][:], wqT[128 * d:128 * (d + 1), :])
                    if f"xq_{t}" not in dma_done:
                        dma_done.add(f"xq_{t}")
                        for d in range(NDT):
                            dma(xq_sb[(d, t)][:],
                                xqT[128 * d:128 * (d + 1),
                                    512 * t:512 * (t + 1)])
                elif kind == "B":
                    t = key[2]
                    if "wk" not in dma_done:
                        dma_done.add("wk")
                        for d in range(NDT):
                            dma(wk_sb[d][:], wkT[128 * d:128 * (d + 1), :])
                    if f"xkv_{t}" not in dma_done:
                        dma_done.add(f"xkv_{t}")
                        for d in range(NDT):
                            dma(xkv_sb[(d, t)][:],
                                xkvT[128 * d:128 * (d + 1),
                                     512 * t:512 * (t + 1)])
                elif kind == "C":
                    tq = key[1] // 4
                    for dk in ("wv", f"xkv_{tq}"):
                        if dk not in dma_done:
                            dma_done.add(dk)
                            if dk == "wv":
                                for d in range(NDT):
                                    dma(wv_sb[d][:],
                                        wvT[128 * d:128 * (d + 1), :])
                            else:
                                for d in range(NDT):
                                    dma(xkv_sb[(d, tq)][:],
                                        xkvT[128 * d:128 * (d + 1),
                                             512 * tq:512 * (tq + 1)])

            dma(bq_sb[:], bq4[:])
            dma(bk_sb[:], bk4[:])
            dma(bv_sb[:], bvb[:])
            dma(mk_sb[:], masks[:])
            AHEAD = 5
            for k in proj_order:
                if first_need[k] <= AHEAD:
                    dma_for(k)
            # later DMAs issue from SP only (Scalar/GpSimd get busy in D)
            dma_engines[:] = [nc.sync]
            wo_dma = [False]

            def dma_wo():
                if not wo_dma[0]:
                    wo_dma[0] = True
                    for e in range(NET):
                        nc.sync.dma_start(wo_sb[e][:],
                                          woT[128 * e:128 * (e + 1), :])
            for k in proj_order:
                dma_for(k)
                if first_need[k] > len(items) // 4:
                    dma_wo()
            dma_wo()

            # softmax-denominator ones columns (static)
            for tt in range(NJB):
                vv = v_sb[tt][:].rearrange("p (h w) -> p h w", w=VH)
                nc.vector.memset(vv[:, :, HD:VH], 1.0)

            # ---- projection-group emitters ----
            def emit_proj(key, prologue):
                kind = key[0]
                if kind == "A" or kind == "B":
                    t, et = (key[1], key[2]) if kind == "A" else (key[2], key[1])
                    wsb, xsb = (wq_sb, xq_sb) if kind == "A" else (wk_sb, xkv_sb)
                    dst = qT if kind == "A" else kT
                    bias = bq_sb if kind == "A" else bk_sb
                    ps = pp.tile([128, 512], F32, tag="pp")
                    for d in range(NDT):
                        nc.tensor.matmul(
                            ps[:], wsb[d][:, 128 * et:128 * (et + 1)],
                            xsb[(d, t)][:],
                            start=(d == 0), stop=(d == NDT - 1))
                    out = dst[et][:, 512 * t:512 * (t + 1)]
                    if prologue:
                        nc.scalar.activation(out, ps[:], AF.Identity,
                                             bias=bias[:, et:et + 1])
                    else:
                        nc.vector.tensor_scalar_add(out, ps[:],
                                                    bias[:, et:et + 1])
                else:
                    tt = key[1]
                    tq, tc_ = tt // 4, tt % 4
                    ps = pp.tile([128, 512], F32, tag="pp")
                    for d in range(NDT):
                        nc.tensor.matmul(
                            ps[:],
                            xkv_sb[(d, tq)][:, 128 * tc_:128 * (tc_ + 1)],
                            wv_sb[d][:], start=(d == 0), stop=(d == NDT - 1))
                    vv = v_sb[tt][:].rearrange("p (h w) -> p h w", w=VH)
                    nc.vector.tensor_tensor(
                        vv[:, :, 0:HD],
                        ps[:].rearrange("p (h w) -> p h w", w=HD),
                        bv_sb[:].rearrange("p (h w) -> p h w", w=HD), ADD)

            # ---- phase D + interleaved fillers ----
            DEPTH = 5
            ot = {}
            issued = {}
            pending = {}
            e_groups = []    # (push_item, ic, do)
            cur_item = [0]
            pq = list(proj_order)

            def emit_e():
                _, ic, do = e_groups.pop(0)
                ps = pp.tile([128, 512], F32, tag="pp")
                for e in range(NET):
                    nc.tensor.matmul(
                        ps[:], wo_sb[e][:, 128 * do:128 * (do + 1)],
                        aT[e][:, 512 * ic:512 * (ic + 1)],
                        start=(e == 0), stop=(e == NET - 1))
                o = stg_pool.tile([128, 512], BF16, tag="stg")
                nc.vector.tensor_scalar_add(o[:], ps[:], 0.0)
                nc.sync.dma_start(
                    outT[128 * do:128 * (do + 1),
                         512 * ic:512 * (ic + 1)], o[:])

            def finish_group(ic, h):
                o = ot.pop((ic, h))
                et, r0 = h // 2, 64 * (h % 2)
                # reciprocal_approx_fast's bitwise seed misreads PSUM inputs;
                # stage the denominator row through SBUF first
                dcp = dv_pool.tile([1, 512], F32, tag="dc")
                nc.vector.tensor_scalar_add(dcp[:], o[64:65, :], 0.0)
                r = dv_pool.tile([1, 512], F32, tag="r")
                nc.vector.reciprocal_approx_fast(r[:], dcp[:])
                rb = dv_pool.tile([64, 512], F32, tag="rb")
                nc.gpsimd.partition_broadcast(rb[:], r[:])
                nc.vector.tensor_tensor(
                    aT[et][r0:r0 + 64, 512 * ic:512 * (ic + 1)],
                    o[0:64, :], rb[:], MUL)
                if h == H_LOC - 1:
                    for do in range(NDT):
                        e_groups.append((cur_item[0], ic, do))

            def emit_pv(n):
                ic, h, pair = items[n]
                pt = pending.pop(n)
                # ascending delta within the pair: group's first-issued PV is
                # always a delta-0 block, so start=True covers [0:512)
                for s in sorted(range(len(pair)), key=lambda q: pair[q][1]):
                    jb, dlt, msk = pair[s]
                    k = issued[(ic, h)] = issued.get((ic, h), 0) + 1
                    nc.tensor.matmul(
                        ot[(ic, h)][:, dlt:512],
                        v_sb[jb][:, VH * h:VH * h + VH],
                        pt[:, 512 * s + dlt:512 * (s + 1)],
                        start=(k == 1), stop=(k == nblocks[ic]),
                        skip_group_check=True)
                if issued[(ic, h)] == nblocks[ic]:
                    finish_group(ic, h)

            # prologue: groups needed by the first AHEAD items
            for k in list(pq):
                if first_need[k] <= AHEAD:
                    emit_proj(k, prologue=True)
                    pq.remove(k)

            for n, (ic, h, pair) in enumerate(items):
                cur_item[0] = n
                # dependency-driven projection groups (with lookahead)
                while pq and first_need[pq[0]] <= n + AHEAD:
                    emit_proj(pq.pop(0), prologue=False)
                if e_groups and n - e_groups[0][0] >= 2:
                    emit_e()
                if (ic, h) not in ot:
                    ot[(ic, h)] = ops.tile([65, 512], F32, tag="ot",
                                           name=f"ot{ic}_{h}")
                et, r0 = h // 2, 64 * (h % 2)
                st = sps.tile([128, 1024], F32, tag="st")
                for s, (jb, dlt, msk) in enumerate(pair):
                    nc.tensor.matmul(
                        st[:, 512 * s + dlt:512 * (s + 1)],
                        kT[et][r0:r0 + 64, 128 * jb:128 * (jb + 1)],
                        qT[et][r0:r0 + 64, 512 * ic + dlt:512 * (ic + 1)],
                        start=True, stop=True)
                pt = pt_pool.tile([128, 1024], BF16, tag="pt", bufs=10)
                lo = pair[0][1]
                hi = 512 * (len(pair) - 1) + 512
                nc.scalar.activation(pt[:, lo:hi], st[:, lo:hi],
                                     AF.Exp, scale=SCALE)
                for s, (jb, dlt, msk) in enumerate(pair):
                    if msk:
                        k = dpos[128 * jb - L - 512 * ic]
                        nc.vector.tensor_tensor(
                            pt[:, 512 * s + dlt:512 * s + dlt + 128],
                            pt[:, 512 * s + dlt:512 * s + dlt + 128],
                            mk_sb[:, 512 * k + dlt:512 * k + dlt + 128], MUL)
                pending[n] = pt
                if n >= DEPTH:
                    emit_pv(n - DEPTH)
            while pq:
                emit_proj(pq.pop(0), prologue=False)
            for n in range(max(0, len(items) - DEPTH), len(items)):
                emit_pv(n)
            while e_groups:
                emit_e()

    nc.compile()
    return nc, deltas


def _prep_core(query, key_value, Wq, bq, Wk, bk, Wv, bv, Wo, c, deltas, L):
    b, half = c // 2, c % 2
    hs = E_LOC * half
    f32, bf16 = np.float32, ml_dtypes.bfloat16
    xqT = np.ascontiguousarray(query[b].T).astype(bf16)
    xkvT = np.ascontiguousarray(key_value[b].T).astype(bf16)
    wqT = np.ascontiguousarray(Wq[hs:hs + E_LOC].T).astype(bf16)
    wkT = np.ascontiguousarray(Wk[hs:hs + E_LOC].T).astype(bf16)
    wvT = np.ascontiguousarray(Wv[hs:hs + E_LOC].T).astype(bf16)
    woT = np.ascontiguousarray(Wo[:, hs:hs + E_LOC].T).astype(bf16)
    bq4 = np.ascontiguousarray(bq[hs:hs + E_LOC].reshape(NET, 128).T, dtype=f32)
    bk4 = np.ascontiguousarray(bk[hs:hs + E_LOC].reshape(NET, 128).T, dtype=f32)
    bvb = np.broadcast_to(bv[hs:hs + E_LOC].reshape(1, E_LOC),
                          (128, E_LOC)).astype(bf16)
    nmask = max(1, len(deltas))
    masks = np.zeros((128, nmask * 512), dtype=np.float32)
    jr = np.arange(128)[:, None]
    ir = np.arange(512)[None, :]
    for k, d in enumerate(deltas):
        masks[:, 512 * k:512 * (k + 1)] = (jr <= ir - d).astype(f32)
    masks = masks.astype(bf16)
    return {"xqT": xqT, "xkvT": xkvT, "wqT": wqT, "wkT": wkT, "wvT": wvT,
            "woT": woT, "bq4": bq4, "bk4": bk4, "bvb": bvb, "masks": masks}


def kernel(query, key_value, Wq, bq, Wk, bk, Wv, bv, Wo, bo, lookahead,
           _trace=False):
    L = int(lookahead)
    if L not in _CACHE:
        _CACHE[L] = _build(L)
    nc, deltas = _CACHE[L]

    args = [np.asarray(a, dtype=np.float32) for a in
            (query, key_value, Wq, bq, Wk, bk, Wv, bv, Wo)]
    in_maps = [_prep_core(*args, c, deltas, L) for c in range(8)]
    res = run_bass_kernel_spmd(nc, in_maps, core_ids=list(range(8)),
                               trace=_trace)
    bo = np.asarray(bo, dtype=np.float32)
    out = np.empty((B, T, D), dtype=np.float32)
    for b in range(B):
        pT = (res.results[2 * b]["outT"].astype(np.float32)
              + res.results[2 * b + 1]["outT"].astype(np.float32))
        out[b] = pT.T + bo[None, :]
    if _trace:
        kernel.last_exec_time_ns = res.exec_time_ns
    return out
